# revision 57
# baseline (speedup 1.0000x reference)
"""Trainium2 Bass kernel for a dense attention block (B=2, T=2048, D=2048, H=32, HD=64).

Sharding: core c in 0..7 handles batch b=c//4 and head-group g=c%4 (8 heads, 512
features). QKV projections are column-parallel per head-group (weights and x
pre-transposed/packed on the host, bf16); RoPE is applied feature-major via a
partition-pair stream_shuffle plus host-built cos/sin maps; causal attention
runs per-head in flash-style 128x512 tiles (scores transposed, exp on ScalarE
without max-subtraction — scores are O(5) — and the softmax denominator comes
from an all-ones block appended to v so PV emits y and broadcast row-sums in
one matmul). The per-core normalized y^T is AllGather'd within each batch's
4-core group in four 512-token quarters (fired as each query chunk finishes;
chunk 0 is processed last to minimize the tail), then the output projection is
token-parallel: each core selects its own 512-token chunk via a one-hot
combine (SPMD-static) and multiplies against the full wo. The host only
shards/packs inputs and concatenates the disjoint output chunks.

All matmuls run in bf16 with fp32 PSUM accumulation. Cost-model estimate:
~480 us/core; engine busy: PE ~377 us, ACT ~168 us, GpSimd ~158 us, DVE ~135 us.
"""

import sys

import numpy as np

sys.path.insert(0, "/opt/trn_rl_repo")

import ml_dtypes

import concourse.bass as bass  # noqa: F401  (registers types)
import concourse.mybir as mybir
import concourse.tile as tile
from concourse import bacc
from concourse import bass_utils

F32 = mybir.dt.float32
BF16 = mybir.dt.bfloat16
bf16 = ml_dtypes.bfloat16

B, T, D = 2, 2048, 2048
H, HD = 32, 64
NCORES = 8
HPC = 8            # heads per core
FPC = HPC * HD     # 512 features per core
KT = D // 128      # 16 k-tiles
NSLAB = 4          # token slabs of 512



def build_nc(n_devices=NCORES, use_collective=True, phases=("qkv", "attn", "wo")):
    nc = bacc.Bacc("TRN2", target_bir_lowering=False, debug=False, num_devices=n_devices)

    # ---- per-core device inputs (host pre-packed, see kernel()) ----
    x_dev = nc.dram_tensor("x_dev", [NSLAB, 128, KT * 512], BF16, kind="ExternalInput")
    wq_dev = nc.dram_tensor("wq_dev", [128, KT * 512], BF16, kind="ExternalInput")
    wk_dev = nc.dram_tensor("wk_dev", [128, KT * 512], BF16, kind="ExternalInput")
    wv_dev = nc.dram_tensor("wv_dev", [128, KT * 512], BF16, kind="ExternalInput")
    wo_dev = nc.dram_tensor("wo_dev", [4, 128, KT * 512], BF16, kind="ExternalInput")
    cos_dev = nc.dram_tensor("cos_dev", [128, T], F32, kind="ExternalInput")
    sin_dev = nc.dram_tensor("sin_dev", [128, T], F32, kind="ExternalInput")
    tri_dev = nc.dram_tensor("tri_dev", [128, 128], BF16, kind="ExternalInput")
    # one-hot row per core (col g = 1.0 for my group index) for static chunk select
    oh_dev = nc.dram_tensor("oh_dev", [128, 4], F32, kind="ExternalInput")

    out_dev = nc.dram_tensor("out_dev", [512, D], F32, kind="ExternalOutput")

    do_attn = "attn" in phases
    do_wo = "wo" in phases and do_attn

    with tile.TileContext(nc) as tc:
        with (
            tc.tile_pool(name="dram", bufs=1, space="DRAM") as dram,
            tc.tile_pool(name="persist", bufs=1) as persist,
            tc.tile_pool(name="psum", bufs=2, space="PSUM") as psum,
            tc.tile_pool(name="probs", bufs=4) as probpool,
            tc.tile_pool(name="ytmp", bufs=3) as ytmp,
        ):
            # DRAM bounce buffers: one per 512-token chunk so each AllGather
            # fires as soon as its chunk completes.
            y_dram = [dram.tile([FPC, 512], BF16, name=f"y_dram{i}")
                      for i in range(4)]
            ag_dram = [dram.tile([4 * FPC, 512], BF16, name=f"ag_dram{i}")
                       for i in range(4)]

            # persistent SBUF across qkv+attention
            qT_sb = persist.tile([128, 4, T], BF16)   # feature-major rope'd q
            kT_sb = persist.tile([128, 4, T], BF16)
            v_aug = persist.tile([128, KT, HPC, 128], BF16)  # [64 v | 64 ones]
            tri_sb = persist.tile([128, 128], BF16)
            nc.gpsimd.memset(v_aug[:, :, :, 64:128], 1.0)

            def do_ag(c):
                if not do_wo:
                    return
                if use_collective:
                    nc.gpsimd.collective_compute(
                        "AllGather",
                        mybir.AluOpType.bypass,
                        replica_groups=[[0, 1, 2, 3], [4, 5, 6, 7]],
                        ins=[y_dram[c].opt()],
                        outs=[ag_dram[c].opt()],
                    )
                else:  # timing stand-in for single-core TimelineSim runs
                    for g in range(4):
                        nc.sync.dma_start(
                            ag_dram[c][FPC * g:FPC * (g + 1), :], y_dram[c][:]
                        )

            # ---- QKV + RoPE (all slabs), attention chunks 1,2 interleaved ----
            with (
                tc.tile_pool(name="wpool", bufs=1) as wpool,
                tc.tile_pool(name="xpool", bufs=2) as xpool,
                tc.tile_pool(name="mpool", bufs=1) as mpool,
                tc.tile_pool(name="rtmp", bufs=2) as rtmp,
            ):
                # DMA emission order matters: slab-0 x and wq first (the first
                # matmuls' operands), then remaining weights, then rope maps.
                x_slab0 = xpool.tile([128, KT, 512], BF16, name="x_slab0")
                xr0 = x_dev[0].rearrange("p (k f) -> p k f", k=KT)
                for kq in range(0, KT, 4):
                    nc.sync.dma_start(x_slab0[:, kq:kq + 4, :], xr0[:, kq:kq + 4, :])

                w_sb = {}
                for nm, wd in (("q", wq_dev), ("k", wk_dev), ("v", wv_dev)):
                    wt = wpool.tile([128, KT, 512], BF16, name=f"w{nm}_sb")
                    wr = wd.ap().rearrange("p (k f) -> p k f", k=KT)
                    for kq in range(0, KT, 4):
                        nc.sync.dma_start(wt[:, kq:kq + 4, :], wr[:, kq:kq + 4, :])
                    w_sb[nm] = wt

                cos_sb = mpool.tile([128, T], F32)
                sin_sb = mpool.tile([128, T], F32)
                nc.sync.dma_start(cos_sb[:], cos_dev[:])
                nc.sync.dma_start(sin_sb[:], sin_dev[:])
                nc.sync.dma_start(tri_sb[:], tri_dev[:])

                swap_mask = [p ^ 1 for p in range(32)]
                for s in range(NSLAB):
                    ts_ = slice(512 * s, 512 * (s + 1))
                    if s == 0:
                        x_sb = x_slab0
                    else:
                        x_sb = xpool.tile([128, KT, 512], BF16, tag="x_slab0")
                        xr = x_dev[s].rearrange("p (k f) -> p k f", k=KT)
                        for kq in range(0, KT, 4):
                            nc.sync.dma_start(x_sb[:, kq:kq + 4, :],
                                              xr[:, kq:kq + 4, :])

                    # q,k feature-major + fused RoPE evacuation
                    for nm, dst in (("q", qT_sb), ("k", kT_sb)):
                        for m in range(4):
                            ps_ = psum.tile([128, 512], F32, tag="qkv")
                            for kk in range(KT):
                                nc.tensor.matmul(
                                    ps_[:],
                                    w_sb[nm][:, kk, 128 * m:128 * (m + 1)],
                                    x_sb[:, kk, :],
                                    start=(kk == 0),
                                    stop=(kk == KT - 1),
                                )
                            shuf = rtmp.tile([128, 512], F32, tag="shuf")
                            nc.vector.stream_shuffle(shuf[:], ps_[:], swap_mask)
                            t1 = rtmp.tile([128, 512], F32, tag="t1")
                            nc.gpsimd.tensor_mul(t1[:], shuf[:], sin_sb[:, ts_])
                            t2 = rtmp.tile([128, 512], F32, tag="t2")
                            nc.vector.tensor_mul(t2[:], ps_[:], cos_sb[:, ts_])
                            nc.vector.tensor_add(dst[:, m, ts_], t1[:], t2[:])

                    # v token-major into v_aug
                    for tt in range(4):
                        tk_idx = 4 * s + tt
                        ps_ = psum.tile([128, 512], F32, tag="qkv")
                        for kk in range(KT):
                            nc.tensor.matmul(
                                ps_[:],
                                x_sb[:, kk, 128 * tt:128 * (tt + 1)],
                                w_sb["v"][:, kk, :],
                                start=(kk == 0),
                                stop=(kk == KT - 1),
                            )
                        nc.vector.tensor_copy(
                            v_aug[:, tk_idx, :, 0:64],
                            ps_.rearrange("p (h f) -> p h f", h=HPC),
                        )

                    if do_attn and s in (1, 2):
                        attn_chunk(nc, qT_sb, kT_sb, v_aug, tri_sb, y_dram,
                                   psum, probpool, ytmp, s)
                        do_ag(s)

            # ---- attention chunks 3 then 0 (rope-era pools closed above, so
            # the wo-phase pools below can coexist with these) ----
            if do_attn:
                attn_chunk(nc, qT_sb, kT_sb, v_aug, tri_sb, y_dram,
                           psum, probpool, ytmp, 3)
                do_ag(3)
                attn_chunk(nc, qT_sb, kT_sb, v_aug, tri_sb, y_dram,
                           psum, probpool, ytmp, 0)
                do_ag(0)

            # ---- token-parallel output projection ----
            if do_wo:
                with (
                    tc.tile_pool(name="wop", bufs=2) as wop,
                    tc.tile_pool(name="ypool", bufs=1) as ypool,
                    tc.tile_pool(name="otmp", bufs=2) as otmp,
                ):
                    oh_sb = ypool.tile([128, 4], F32)
                    nc.sync.dma_start(oh_sb[:], oh_dev[:])
                    yacc = ypool.tile([128, KT, 512], BF16)
                    for i, g in enumerate([1, 2, 3, 0]):  # chunk-arrival order
                        cand = ypool.tile([128, KT, 512], BF16, tag="cand",
                                          bufs=2, name=f"cand{g}")
                        nc.sync.dma_start(
                            cand[:], ag_dram[g].rearrange("(k p) t -> p k t", p=128)
                        )
                        # one-hot select, split across DVE / GpSimd halves
                        ks = slice(0, 10)
                        nc.vector.scalar_tensor_tensor(
                            yacc[:, ks, :], cand[:, ks, :], oh_sb[:, g:g + 1],
                            yacc[:, ks, :],
                            op0=mybir.AluOpType.mult,
                            op1=(mybir.AluOpType.bypass if i == 0
                                 else mybir.AluOpType.add),
                        )
                        ks = slice(10, 16)
                        gt = ypool.tile([128, 6, 512], BF16, tag="gsel", bufs=2)
                        nc.gpsimd.tensor_scalar_mul(
                            gt[:], cand[:, ks, :], oh_sb[:, g:g + 1]
                        )
                        if i == 0:
                            nc.gpsimd.tensor_copy(yacc[:, ks, :], gt[:])
                        else:
                            nc.gpsimd.tensor_add(yacc[:, ks, :], yacc[:, ks, :], gt[:])

                    for n in range(4):
                        wo_sb = wop.tile([128, KT, 512], BF16)
                        nc.sync.dma_start(
                            wo_sb[:], wo_dev[n].rearrange("p (k f) -> p k f", k=KT)
                        )
                        for m in range(4):
                            opsum = psum.tile([128, 512], F32, tag="qkv")
                            for kk in range(KT):
                                nc.tensor.matmul(
                                    opsum[:],
                                    yacc[:, kk, 128 * m:128 * (m + 1)],
                                    wo_sb[:, kk, :],
                                    start=(kk == 0),
                                    stop=(kk == KT - 1),
                                )
                            osb = otmp.tile([128, 512], F32)
                            nc.scalar.copy(osb[:], opsum[:])
                            nc.sync.dma_start(
                                out_dev[128 * m:128 * (m + 1),
                                        512 * n:512 * (n + 1)],
                                osb[:],
                            )

    nc.compile()
    return nc


def attn_chunk(nc, qT_sb, kT_sb, v_aug, tri_sb, y_dram, psum, probpool,
               ytmp, c):
    q0 = 512 * c
    cs = slice(q0, q0 + 512)
    ntk = 4 * (c + 1)
    for h in range(HPC):
        mt, hh = divmod(h, 2)
        hs = slice(64 * hh, 64 * (hh + 1))
        ypsum = psum.tile([128, 512], F32, tag="y")

        # score tiles in groups of 2 (one 2-bank psum + one exp each);
        # the last 4 tiles of the chunk carry the causal boundary.
        for g0 in range(0, ntk, 2):
            sps = psum.tile([128, 1024], F32, tag="s2")
            for j in range(2):
                tk = g0 + j
                nc.tensor.matmul(
                    sps[:, 512 * j:512 * (j + 1)],
                    kT_sb[hs, mt, 128 * tk:128 * (tk + 1)],
                    qT_sb[hs, mt, cs],
                    start=True,
                    stop=True,
                )
            pr = probpool.tile([128, 1024], BF16, tag="pr")
            if g0 < 4 * c:
                # fully below the diagonal: plain exp on both halves
                nc.scalar.activation(
                    pr[:], sps[:], mybir.ActivationFunctionType.Exp,
                    scale=0.125,
                )
            else:
                for j in range(2):
                    tk = g0 + j
                    rel = 128 * tk - q0
                    lo = 512 * j
                    if rel > 0:
                        nc.gpsimd.memset(pr[:, lo:lo + rel], 0.0)
                    nc.scalar.activation(
                        pr[:, lo + rel:lo + 512],
                        sps[:, lo + rel:lo + 512],
                        mybir.ActivationFunctionType.Exp, scale=0.125,
                    )
                    nc.gpsimd.tensor_mul(
                        pr[:, lo + rel:lo + rel + 128],
                        pr[:, lo + rel:lo + rel + 128],
                        tri_sb[:],
                    )
            for j in range(2):
                tk = g0 + j
                nc.tensor.matmul(
                    ypsum[:],
                    v_aug[:, tk, h, :],
                    pr[:, 512 * j:512 * (j + 1)],
                    start=(tk == 0),
                    stop=(tk == ntk - 1),
                )

        recip = ytmp.tile([128, 512], F32, tag="recip")
        nc.vector.reciprocal(recip[64:128, :], ypsum[64:128, :])
        ynorm = ytmp.tile([64, 512], BF16, tag="ynorm")
        nc.vector.tensor_mul(ynorm[:], ypsum[0:64, :], recip[64:128, :])
        nc.sync.dma_start(y_dram[c][64 * h:64 * (h + 1), :], ynorm[:])


_NC_CACHE = {}


def _get_nc():
    if "nc" not in _NC_CACHE:
        _NC_CACHE["nc"] = build_nc()
    return _NC_CACHE["nc"]


def _pack_weightT(w):
    """(512, D) weight shard -> (128, KT*512) bf16 SBUF image of w.T k-tiles."""
    wT = np.ascontiguousarray(w.T)                       # (D, 512)
    r = wT.reshape(KT, 128, 512).transpose(1, 0, 2)      # (128, KT, 512)
    return np.ascontiguousarray(r.reshape(128, KT * 512).astype(bf16))


def make_in_maps(x, freqs_cos, freqs_sin, wq, wk, wv, wo):
    x = np.asarray(x, dtype=np.float32)
    freqs_cos = np.asarray(freqs_cos, dtype=np.float32)
    freqs_sin = np.asarray(freqs_sin, dtype=np.float32)
    wq, wk, wv, wo = (np.asarray(w, dtype=np.float32) for w in (wq, wk, wv, wo))

    # shared host-side packs
    p_idx = np.arange(128)
    j_idx = (p_idx % 64) // 2
    cosmap = freqs_cos.T[j_idx, :].astype(np.float32)               # (128, T)
    sinmap = (np.where(p_idx % 2 == 0, -1.0, 1.0)[:, None]
              * freqs_sin.T[j_idx, :]).astype(np.float32)
    tri = np.triu(np.ones((128, 128), np.float32)).astype(bf16)

    # x pack per batch: (NSLAB, 128, KT*512) where [s, p, kk*512 + t] = x[b, 512s+t, 128kk+p]
    x_pack = {}
    for b in range(B):
        xT = np.ascontiguousarray(x[b].T)                           # (D, T)
        r = xT.reshape(KT, 128, NSLAB, 512)                         # kk, p, s, t
        x_pack[b] = np.ascontiguousarray(
            r.transpose(2, 1, 0, 3).reshape(NSLAB, 128, KT * 512).astype(bf16)
        )

    # wo pack: (4, 128, KT*512) where [n, p, kk*512+f] = wo[512n+f, 128kk+p]
    woT = np.ascontiguousarray(wo.T)                                # (D, D) in-feat major
    r = woT.reshape(KT, 128, 4, 512)
    wo_pack = np.ascontiguousarray(
        r.transpose(2, 1, 0, 3).reshape(4, 128, KT * 512).astype(bf16)
    )

    in_maps = []
    for c in range(NCORES):
        b, g = divmod(c, 4)
        fs = slice(FPC * g, FPC * (g + 1))
        in_maps.append(
            {
                "x_dev": x_pack[b],
                "wq_dev": _pack_weightT(wq[fs]),
                "wk_dev": _pack_weightT(wk[fs]),
                "wv_dev": _pack_weightT(wv[fs]),
                "wo_dev": wo_pack,
                "cos_dev": cosmap,
                "sin_dev": sinmap,
                "tri_dev": tri,
                "oh_dev": np.tile(np.eye(4, dtype=np.float32)[g], (128, 1)),
            }
        )
    return in_maps


def kernel(x, freqs_cos, freqs_sin, wq, wk, wv, wo):
    nc = _get_nc()
    in_maps = make_in_maps(x, freqs_cos, freqs_sin, wq, wk, wv, wo)

    res = bass_utils.run_bass_kernel_spmd(
        nc, in_maps, core_ids=list(range(NCORES)), **_NC_CACHE.get("run_kwargs", {})
    )
    _NC_CACHE["last_result"] = res

    out = np.empty((B, T, D), dtype=np.float32)
    for c in range(NCORES):
        b, g = divmod(c, 4)
        out[b, 512 * g:512 * (g + 1), :] = res.results[c]["out_dev"]
    return out


# revision 58
# speedup vs baseline: 1.0144x; 1.0144x over previous
"""Trainium2 Bass kernel for a dense attention block (B=2, T=2048, D=2048, H=32, HD=64).

Sharding: core c in 0..7 handles batch b=c//4 and head-group g=c%4 (8 heads, 512
features). QKV projections are column-parallel per head-group (weights and x
pre-transposed/packed on the host, bf16); RoPE is applied feature-major via a
partition-pair stream_shuffle plus host-built cos/sin maps; causal attention
runs per-head in flash-style 128x512 tiles (scores transposed, exp on ScalarE
without max-subtraction — scores are O(5) — and the softmax denominator comes
from an all-ones block appended to v so PV emits y and broadcast row-sums in
one matmul). The per-core normalized y^T is AllGather'd within each batch's
4-core group in four 512-token quarters (fired as each query chunk finishes;
chunk 0 is processed last to minimize the tail), then the output projection is
token-parallel: each core selects its own 512-token chunk via a one-hot
combine (SPMD-static) and multiplies against the full wo. The host only
shards/packs inputs and concatenates the disjoint output chunks.

All matmuls run in bf16 with fp32 PSUM accumulation. Cost-model estimate:
~480 us/core; engine busy: PE ~377 us, ACT ~168 us, GpSimd ~158 us, DVE ~135 us.
"""

import sys

import numpy as np

sys.path.insert(0, "/opt/trn_rl_repo")

import ml_dtypes

import concourse.bass as bass  # noqa: F401  (registers types)
import concourse.mybir as mybir
import concourse.tile as tile
from concourse import bacc
from concourse import bass_utils

F32 = mybir.dt.float32
BF16 = mybir.dt.bfloat16
bf16 = ml_dtypes.bfloat16

B, T, D = 2, 2048, 2048
H, HD = 32, 64
NCORES = 8
HPC = 8            # heads per core
FPC = HPC * HD     # 512 features per core
KT = D // 128      # 16 k-tiles
NSLAB = 4          # token slabs of 512



def build_nc(n_devices=NCORES, use_collective=True, phases=("qkv", "attn", "wo")):
    nc = bacc.Bacc("TRN2", target_bir_lowering=False, debug=False, num_devices=n_devices)

    # ---- per-core device inputs (host pre-packed, see kernel()) ----
    x_dev = nc.dram_tensor("x_dev", [NSLAB, 128, KT * 512], BF16, kind="ExternalInput")
    wq_dev = nc.dram_tensor("wq_dev", [128, KT * 512], BF16, kind="ExternalInput")
    wk_dev = nc.dram_tensor("wk_dev", [128, KT * 512], BF16, kind="ExternalInput")
    wv_dev = nc.dram_tensor("wv_dev", [128, KT * 512], BF16, kind="ExternalInput")
    wo_dev = nc.dram_tensor("wo_dev", [4, 128, KT * 512], BF16, kind="ExternalInput")
    cos_dev = nc.dram_tensor("cos_dev", [128, T], F32, kind="ExternalInput")
    sin_dev = nc.dram_tensor("sin_dev", [128, T], F32, kind="ExternalInput")
    tri_dev = nc.dram_tensor("tri_dev", [128, 128], BF16, kind="ExternalInput")
    # one-hot row per core (col g = 1.0 for my group index) for static chunk select
    oh_dev = nc.dram_tensor("oh_dev", [128, 4], F32, kind="ExternalInput")

    out_dev = nc.dram_tensor("out_dev", [512, D], F32, kind="ExternalOutput")

    do_attn = "attn" in phases
    do_wo = "wo" in phases and do_attn

    with tile.TileContext(nc) as tc:
        with (
            tc.tile_pool(name="dram", bufs=1, space="DRAM") as dram,
            tc.tile_pool(name="persist", bufs=1) as persist,
            tc.tile_pool(name="psum", bufs=2, space="PSUM") as psum,
            tc.tile_pool(name="probs", bufs=4) as probpool,
            tc.tile_pool(name="ytmp", bufs=3) as ytmp,
        ):
            # DRAM bounce buffers: one per 512-token chunk so each AllGather
            # fires as soon as its chunk completes.
            y_dram = [dram.tile([FPC, 512], BF16, name=f"y_dram{i}")
                      for i in range(4)]
            ag_dram = [dram.tile([4 * FPC, 512], BF16, name=f"ag_dram{i}")
                       for i in range(4)]

            # persistent SBUF across qkv+attention
            qT_sb = persist.tile([128, 4, T], BF16)   # feature-major rope'd q
            kT_sb = persist.tile([128, 4, T], BF16)
            v_aug = persist.tile([128, KT, HPC, 128], BF16)  # [64 v | 64 ones]
            tri_sb = persist.tile([128, 128], BF16)
            nc.gpsimd.memset(v_aug[:, :, :, 64:128], 1.0)

            def do_ag(c):
                if not do_wo:
                    return
                if use_collective:
                    nc.gpsimd.collective_compute(
                        "AllGather",
                        mybir.AluOpType.bypass,
                        replica_groups=[[0, 1, 2, 3], [4, 5, 6, 7]],
                        ins=[y_dram[c].opt()],
                        outs=[ag_dram[c].opt()],
                    )
                else:  # timing stand-in for single-core TimelineSim runs
                    for g in range(4):
                        nc.sync.dma_start(
                            ag_dram[c][FPC * g:FPC * (g + 1), :], y_dram[c][:]
                        )

            # ---- QKV + RoPE (all slabs), attention chunks 1,2 interleaved ----
            with (
                tc.tile_pool(name="wpool", bufs=1) as wpool,
                tc.tile_pool(name="xpool", bufs=2) as xpool,
                tc.tile_pool(name="mpool", bufs=1) as mpool,
                tc.tile_pool(name="rtmp", bufs=2) as rtmp,
            ):
                # DMA emission order matters: slab-0 x and wq first (the first
                # matmuls' operands), then remaining weights, then rope maps.
                x_slab0 = xpool.tile([128, KT, 512], BF16, name="x_slab0")
                xr0 = x_dev[0].rearrange("p (k f) -> p k f", k=KT)
                for kq in range(0, KT, 4):
                    nc.sync.dma_start(x_slab0[:, kq:kq + 4, :], xr0[:, kq:kq + 4, :])

                w_sb = {}
                for nm, wd in (("q", wq_dev), ("k", wk_dev), ("v", wv_dev)):
                    wt = wpool.tile([128, KT, 512], BF16, name=f"w{nm}_sb")
                    wr = wd.ap().rearrange("p (k f) -> p k f", k=KT)
                    for kq in range(0, KT, 4):
                        nc.sync.dma_start(wt[:, kq:kq + 4, :], wr[:, kq:kq + 4, :])
                    w_sb[nm] = wt

                cos_sb = mpool.tile([128, T], F32)
                sin_sb = mpool.tile([128, T], F32)
                nc.sync.dma_start(cos_sb[:], cos_dev[:])
                nc.sync.dma_start(sin_sb[:], sin_dev[:])
                nc.sync.dma_start(tri_sb[:], tri_dev[:])

                swap_mask = [p ^ 1 for p in range(32)]
                for s in range(NSLAB):
                    ts_ = slice(512 * s, 512 * (s + 1))
                    if s == 0:
                        x_sb = x_slab0
                    else:
                        x_sb = xpool.tile([128, KT, 512], BF16, tag="x_slab0")
                        xr = x_dev[s].rearrange("p (k f) -> p k f", k=KT)
                        for kq in range(0, KT, 4):
                            nc.sync.dma_start(x_sb[:, kq:kq + 4, :],
                                              xr[:, kq:kq + 4, :])

                    # q,k feature-major + fused RoPE evacuation
                    for nm, dst in (("q", qT_sb), ("k", kT_sb)):
                        for m in range(4):
                            ps_ = psum.tile([128, 512], F32, tag="qkv")
                            for kk in range(KT):
                                nc.tensor.matmul(
                                    ps_[:],
                                    w_sb[nm][:, kk, 128 * m:128 * (m + 1)],
                                    x_sb[:, kk, :],
                                    start=(kk == 0),
                                    stop=(kk == KT - 1),
                                )
                            shuf = rtmp.tile([128, 512], F32, tag="shuf")
                            nc.vector.stream_shuffle(shuf[:], ps_[:], swap_mask)
                            t1 = rtmp.tile([128, 512], F32, tag="t1")
                            nc.gpsimd.tensor_mul(t1[:], shuf[:], sin_sb[:, ts_])
                            t2 = rtmp.tile([128, 512], F32, tag="t2")
                            nc.vector.tensor_mul(t2[:], ps_[:], cos_sb[:, ts_])
                            nc.vector.tensor_add(dst[:, m, ts_], t1[:], t2[:])

                    # v token-major into v_aug
                    for tt in range(4):
                        tk_idx = 4 * s + tt
                        ps_ = psum.tile([128, 512], F32, tag="qkv")
                        for kk in range(KT):
                            nc.tensor.matmul(
                                ps_[:],
                                x_sb[:, kk, 128 * tt:128 * (tt + 1)],
                                w_sb["v"][:, kk, :],
                                start=(kk == 0),
                                stop=(kk == KT - 1),
                            )
                        nc.vector.tensor_copy(
                            v_aug[:, tk_idx, :, 0:64],
                            ps_.rearrange("p (h f) -> p h f", h=HPC),
                        )

                    if do_attn and s in (1, 2):
                        attn_chunk(nc, qT_sb, kT_sb, v_aug, tri_sb, y_dram,
                                   psum, probpool, ytmp, s)
                        do_ag(s)

            # ---- attention chunks 3 then 0 (rope-era pools closed above, so
            # the wo-phase pools below can coexist with these) ----
            if do_attn:
                attn_chunk(nc, qT_sb, kT_sb, v_aug, tri_sb, y_dram,
                           psum, probpool, ytmp, 3)
                do_ag(3)
                attn_chunk(nc, qT_sb, kT_sb, v_aug, tri_sb, y_dram,
                           psum, probpool, ytmp, 0)
                do_ag(0)

            # ---- token-parallel output projection ----
            if do_wo:
                with (
                    tc.tile_pool(name="wop", bufs=2) as wop,
                    tc.tile_pool(name="ypool", bufs=1) as ypool,
                    tc.tile_pool(name="otmp", bufs=2) as otmp,
                ):
                    oh_sb = ypool.tile([128, 4], F32)
                    nc.sync.dma_start(oh_sb[:], oh_dev[:])
                    yacc = ypool.tile([128, KT, 512], BF16)
                    for i, g in enumerate([1, 2, 3, 0]):  # chunk-arrival order
                        cand = ypool.tile([128, KT, 512], BF16, tag="cand",
                                          bufs=2, name=f"cand{g}")
                        nc.sync.dma_start(
                            cand[:], ag_dram[g].rearrange("(k p) t -> p k t", p=128)
                        )
                        # one-hot select, split across DVE / GpSimd halves
                        ks = slice(0, 10)
                        nc.vector.scalar_tensor_tensor(
                            yacc[:, ks, :], cand[:, ks, :], oh_sb[:, g:g + 1],
                            yacc[:, ks, :],
                            op0=mybir.AluOpType.mult,
                            op1=(mybir.AluOpType.bypass if i == 0
                                 else mybir.AluOpType.add),
                        )
                        ks = slice(10, 16)
                        gt = ypool.tile([128, 6, 512], BF16, tag="gsel", bufs=2)
                        nc.gpsimd.tensor_scalar_mul(
                            gt[:], cand[:, ks, :], oh_sb[:, g:g + 1]
                        )
                        if i == 0:
                            nc.gpsimd.tensor_copy(yacc[:, ks, :], gt[:])
                        else:
                            nc.gpsimd.tensor_add(yacc[:, ks, :], yacc[:, ks, :], gt[:])

                    for n in range(4):
                        wo_sb = wop.tile([128, KT, 512], BF16)
                        nc.sync.dma_start(
                            wo_sb[:], wo_dev[n].rearrange("p (k f) -> p k f", k=KT)
                        )
                        for m in range(4):
                            opsum = psum.tile([128, 512], F32, tag="qkv")
                            for kk in range(KT):
                                nc.tensor.matmul(
                                    opsum[:],
                                    yacc[:, kk, 128 * m:128 * (m + 1)],
                                    wo_sb[:, kk, :],
                                    start=(kk == 0),
                                    stop=(kk == KT - 1),
                                )
                            osb = otmp.tile([128, 512], F32)
                            nc.scalar.copy(osb[:], opsum[:])
                            nc.sync.dma_start(
                                out_dev[128 * m:128 * (m + 1),
                                        512 * n:512 * (n + 1)],
                                osb[:],
                            )

    nc.compile()
    return nc


def attn_chunk(nc, qT_sb, kT_sb, v_aug, tri_sb, y_dram, psum, probpool,
               ytmp, c):
    """Causal attention for query chunk c, all 8 heads, processed in pairs.

    The two heads of a pair sit in partition rows [0:64) and [64:128) of the
    same qT/kT feature tile, so their K=64 score matmuls land on different PE
    row-groups (tile_position auto-derived from base partitions) and execute
    concurrently. Each (128, 1024) scores psum holds one k-tile for BOTH
    heads; one exp covers both."""
    q0 = 512 * c
    cs = slice(q0, q0 + 512)
    ntk = 4 * (c + 1)
    for hp in range(4):
        hA, hB = 2 * hp, 2 * hp + 1
        ypsA = psum.tile([128, 512], F32, tag="y", name=f"ypsA_{c}_{hp}")
        ypsB = psum.tile([128, 512], F32, tag="y", name=f"ypsB_{c}_{hp}")
        for tk in range(ntk):
            ks = slice(128 * tk, 128 * (tk + 1))
            sAB = psum.tile([128, 1024], F32, tag="s2")
            nc.tensor.matmul(sAB[:, 0:512], kT_sb[0:64, hp, ks],
                             qT_sb[0:64, hp, cs], start=True, stop=True)
            nc.tensor.matmul(sAB[:, 512:1024], kT_sb[64:128, hp, ks],
                             qT_sb[64:128, hp, cs], start=True, stop=True)
            pr = probpool.tile([128, 1024], BF16, tag="pr")
            rel = 128 * tk - q0
            if rel < 0:
                nc.scalar.activation(
                    pr[:], sAB[:], mybir.ActivationFunctionType.Exp, scale=0.125,
                )
            else:
                for lo in (0, 512):
                    if rel > 0:
                        nc.gpsimd.memset(pr[:, lo:lo + rel], 0.0)
                    nc.scalar.activation(
                        pr[:, lo + rel:lo + 512], sAB[:, lo + rel:lo + 512],
                        mybir.ActivationFunctionType.Exp, scale=0.125,
                    )
                    nc.gpsimd.tensor_mul(
                        pr[:, lo + rel:lo + rel + 128],
                        pr[:, lo + rel:lo + rel + 128],
                        tri_sb[:],
                    )
            first, last = tk == 0, tk == ntk - 1
            nc.tensor.matmul(ypsA[:], v_aug[:, tk, hA, :], pr[:, 0:512],
                             start=first, stop=last)
            nc.tensor.matmul(ypsB[:], v_aug[:, tk, hB, :], pr[:, 512:1024],
                             start=first, stop=last)

        for h, yps in ((hA, ypsA), (hB, ypsB)):
            recip = ytmp.tile([128, 512], F32, tag="recip")
            nc.vector.reciprocal(recip[64:128, :], yps[64:128, :])
            ynorm = ytmp.tile([64, 512], BF16, tag="ynorm")
            nc.vector.tensor_mul(ynorm[:], yps[0:64, :], recip[64:128, :])
            nc.sync.dma_start(y_dram[c][64 * h:64 * (h + 1), :], ynorm[:])


_NC_CACHE = {}


def _get_nc():
    if "nc" not in _NC_CACHE:
        _NC_CACHE["nc"] = build_nc()
    return _NC_CACHE["nc"]


def _pack_weightT(w):
    """(512, D) weight shard -> (128, KT*512) bf16 SBUF image of w.T k-tiles."""
    wT = np.ascontiguousarray(w.T)                       # (D, 512)
    r = wT.reshape(KT, 128, 512).transpose(1, 0, 2)      # (128, KT, 512)
    return np.ascontiguousarray(r.reshape(128, KT * 512).astype(bf16))


def make_in_maps(x, freqs_cos, freqs_sin, wq, wk, wv, wo):
    x = np.asarray(x, dtype=np.float32)
    freqs_cos = np.asarray(freqs_cos, dtype=np.float32)
    freqs_sin = np.asarray(freqs_sin, dtype=np.float32)
    wq, wk, wv, wo = (np.asarray(w, dtype=np.float32) for w in (wq, wk, wv, wo))

    # shared host-side packs
    p_idx = np.arange(128)
    j_idx = (p_idx % 64) // 2
    cosmap = freqs_cos.T[j_idx, :].astype(np.float32)               # (128, T)
    sinmap = (np.where(p_idx % 2 == 0, -1.0, 1.0)[:, None]
              * freqs_sin.T[j_idx, :]).astype(np.float32)
    tri = np.triu(np.ones((128, 128), np.float32)).astype(bf16)

    # x pack per batch: (NSLAB, 128, KT*512) where [s, p, kk*512 + t] = x[b, 512s+t, 128kk+p]
    x_pack = {}
    for b in range(B):
        xT = np.ascontiguousarray(x[b].T)                           # (D, T)
        r = xT.reshape(KT, 128, NSLAB, 512)                         # kk, p, s, t
        x_pack[b] = np.ascontiguousarray(
            r.transpose(2, 1, 0, 3).reshape(NSLAB, 128, KT * 512).astype(bf16)
        )

    # wo pack: (4, 128, KT*512) where [n, p, kk*512+f] = wo[512n+f, 128kk+p]
    woT = np.ascontiguousarray(wo.T)                                # (D, D) in-feat major
    r = woT.reshape(KT, 128, 4, 512)
    wo_pack = np.ascontiguousarray(
        r.transpose(2, 1, 0, 3).reshape(4, 128, KT * 512).astype(bf16)
    )

    in_maps = []
    for c in range(NCORES):
        b, g = divmod(c, 4)
        fs = slice(FPC * g, FPC * (g + 1))
        in_maps.append(
            {
                "x_dev": x_pack[b],
                "wq_dev": _pack_weightT(wq[fs]),
                "wk_dev": _pack_weightT(wk[fs]),
                "wv_dev": _pack_weightT(wv[fs]),
                "wo_dev": wo_pack,
                "cos_dev": cosmap,
                "sin_dev": sinmap,
                "tri_dev": tri,
                "oh_dev": np.tile(np.eye(4, dtype=np.float32)[g], (128, 1)),
            }
        )
    return in_maps


def kernel(x, freqs_cos, freqs_sin, wq, wk, wv, wo):
    nc = _get_nc()
    in_maps = make_in_maps(x, freqs_cos, freqs_sin, wq, wk, wv, wo)

    res = bass_utils.run_bass_kernel_spmd(
        nc, in_maps, core_ids=list(range(NCORES)), **_NC_CACHE.get("run_kwargs", {})
    )
    _NC_CACHE["last_result"] = res

    out = np.empty((B, T, D), dtype=np.float32)
    for c in range(NCORES):
        b, g = divmod(c, 4)
        out[b, 512 * g:512 * (g + 1), :] = res.results[c]["out_dev"]
    return out


# revision 63
# speedup vs baseline: 1.0412x; 1.0264x over previous
"""Trainium2 Bass kernel for a dense attention block (B=2, T=2048, D=2048, H=32, HD=64).

Sharding: core c in 0..7 handles batch b=c//4 and head-group g=c%4 (8 heads, 512
features). QKV projections are column-parallel per head-group (weights and x
pre-transposed/packed on the host, bf16); RoPE is applied feature-major via a
partition-pair stream_shuffle plus host-built cos/sin maps; causal attention
runs per-head in flash-style 128x512 tiles (scores transposed, exp on ScalarE
without max-subtraction — scores are O(5) — and the softmax denominator comes
from an all-ones block appended to v so PV emits y and broadcast row-sums in
one matmul). The per-core normalized y^T is AllGather'd within each batch's
4-core group in four 512-token quarters (fired as each query chunk finishes;
chunk 0 is processed last to minimize the tail), then the output projection is
token-parallel: each core selects its own 512-token chunk via a one-hot
combine (SPMD-static) and multiplies against the full wo. The host only
shards/packs inputs and concatenates the disjoint output chunks.

All matmuls run in bf16 with fp32 PSUM accumulation. Cost-model estimate:
~480 us/core; engine busy: PE ~377 us, ACT ~168 us, GpSimd ~158 us, DVE ~135 us.
"""

import sys

import numpy as np

sys.path.insert(0, "/opt/trn_rl_repo")

import ml_dtypes

import concourse.bass as bass  # noqa: F401  (registers types)
import concourse.mybir as mybir
import concourse.tile as tile
from concourse import bacc
from concourse import bass_utils

F32 = mybir.dt.float32
BF16 = mybir.dt.bfloat16
bf16 = ml_dtypes.bfloat16

B, T, D = 2, 2048, 2048
H, HD = 32, 64
NCORES = 8
HPC = 8            # heads per core
FPC = HPC * HD     # 512 features per core
KT = D // 128      # 16 k-tiles
NSLAB = 4          # token slabs of 512



def build_nc(n_devices=NCORES, use_collective=True, phases=("qkv", "attn", "wo")):
    nc = bacc.Bacc("TRN2", target_bir_lowering=False, debug=False, num_devices=n_devices)

    # ---- per-core device inputs (host pre-packed, see kernel()) ----
    x_dev = nc.dram_tensor("x_dev", [NSLAB, 128, KT * 512], BF16, kind="ExternalInput")
    wq_dev = nc.dram_tensor("wq_dev", [128, KT * 512], BF16, kind="ExternalInput")
    wk_dev = nc.dram_tensor("wk_dev", [128, KT * 512], BF16, kind="ExternalInput")
    wv_dev = nc.dram_tensor("wv_dev", [128, KT * 512], BF16, kind="ExternalInput")
    wo_dev = nc.dram_tensor("wo_dev", [4, 128, KT * 512], BF16, kind="ExternalInput")
    cos_dev = nc.dram_tensor("cos_dev", [128, T], F32, kind="ExternalInput")
    sin_dev = nc.dram_tensor("sin_dev", [128, T], F32, kind="ExternalInput")
    tri_dev = nc.dram_tensor("tri_dev", [128, 256], BF16, kind="ExternalInput")
    # one-hot row per core (col g = 1.0 for my group index) for static chunk select
    oh_dev = nc.dram_tensor("oh_dev", [128, 4], F32, kind="ExternalInput")

    out_dev = nc.dram_tensor("out_dev", [512, D], F32, kind="ExternalOutput")

    do_attn = "attn" in phases
    do_wo = "wo" in phases and do_attn

    with tile.TileContext(nc) as tc:
        with (
            tc.tile_pool(name="dram", bufs=1, space="DRAM") as dram,
            tc.tile_pool(name="persist", bufs=1) as persist,
            tc.tile_pool(name="psum", bufs=2, space="PSUM") as psum,
            tc.tile_pool(name="probs", bufs=4) as probpool,
            tc.tile_pool(name="ytmp", bufs=3) as ytmp,
        ):
            # DRAM bounce buffers: one per 512-token chunk so each AllGather
            # fires as soon as its chunk completes.
            y_dram = [dram.tile([FPC, 512], BF16, name=f"y_dram{i}")
                      for i in range(4)]
            ag_dram = [dram.tile([4 * FPC, 512], BF16, name=f"ag_dram{i}")
                       for i in range(4)]

            # persistent SBUF across qkv+attention
            qT_sb = persist.tile([128, 4, T], BF16)   # feature-major rope'd q
            kT_sb = persist.tile([128, 4, T], BF16)
            v_aug = persist.tile([128, KT, HPC, 128], BF16)  # [64 v | 64 ones]
            tri_sb = persist.tile([128, 256], BF16)  # [tri | tri] doubled
            nc.gpsimd.memset(v_aug[:, :, :, 64:128], 1.0)

            def do_ag(c):
                if not do_wo:
                    return
                if use_collective:
                    nc.gpsimd.collective_compute(
                        "AllGather",
                        mybir.AluOpType.bypass,
                        replica_groups=[[0, 1, 2, 3], [4, 5, 6, 7]],
                        ins=[y_dram[c].opt()],
                        outs=[ag_dram[c].opt()],
                    )
                else:  # timing stand-in for single-core TimelineSim runs
                    for g in range(4):
                        nc.sync.dma_start(
                            ag_dram[c][FPC * g:FPC * (g + 1), :], y_dram[c][:]
                        )

            # ---- QKV + RoPE (all slabs), attention chunks 1,2 interleaved ----
            with (
                tc.tile_pool(name="wpool", bufs=1) as wpool,
                tc.tile_pool(name="xpool", bufs=2) as xpool,
                tc.tile_pool(name="mpool", bufs=1) as mpool,
                tc.tile_pool(name="rtmp", bufs=2) as rtmp,
            ):
                # DMA emission order matters: slab-0 x and wq first (the first
                # matmuls' operands), then remaining weights, then rope maps.
                x_slab0 = xpool.tile([128, KT, 512], BF16, name="x_slab0")
                xr0 = x_dev[0].rearrange("p (k f) -> p k f", k=KT)
                for kq in range(0, KT, 4):
                    nc.sync.dma_start(x_slab0[:, kq:kq + 4, :], xr0[:, kq:kq + 4, :])

                w_sb = {}
                for nm, wd in (("q", wq_dev), ("k", wk_dev), ("v", wv_dev)):
                    wt = wpool.tile([128, KT, 512], BF16, name=f"w{nm}_sb")
                    wr = wd.ap().rearrange("p (k f) -> p k f", k=KT)
                    for kq in range(0, KT, 4):
                        nc.sync.dma_start(wt[:, kq:kq + 4, :], wr[:, kq:kq + 4, :])
                    w_sb[nm] = wt

                cos_sb = mpool.tile([128, T], F32)
                sin_sb = mpool.tile([128, T], F32)
                nc.sync.dma_start(cos_sb[:], cos_dev[:])
                nc.sync.dma_start(sin_sb[:], sin_dev[:])
                nc.sync.dma_start(tri_sb[:], tri_dev[:])

                swap_mask = [p ^ 1 for p in range(32)]
                for s in range(NSLAB):
                    ts_ = slice(512 * s, 512 * (s + 1))
                    if s == 0:
                        x_sb = x_slab0
                    else:
                        x_sb = xpool.tile([128, KT, 512], BF16, tag="x_slab0")
                        xr = x_dev[s].rearrange("p (k f) -> p k f", k=KT)
                        for kq in range(0, KT, 4):
                            nc.sync.dma_start(x_sb[:, kq:kq + 4, :],
                                              xr[:, kq:kq + 4, :])

                    # q,k feature-major + fused RoPE evacuation
                    for nm, dst in (("q", qT_sb), ("k", kT_sb)):
                        for m in range(4):
                            ps_ = psum.tile([128, 512], F32, tag="qkv")
                            for kk in range(KT):
                                nc.tensor.matmul(
                                    ps_[:],
                                    w_sb[nm][:, kk, 128 * m:128 * (m + 1)],
                                    x_sb[:, kk, :],
                                    start=(kk == 0),
                                    stop=(kk == KT - 1),
                                )
                            shuf = rtmp.tile([128, 512], F32, tag="shuf")
                            nc.vector.stream_shuffle(shuf[:], ps_[:], swap_mask)
                            t1 = rtmp.tile([128, 512], F32, tag="t1")
                            nc.gpsimd.tensor_mul(t1[:], shuf[:], sin_sb[:, ts_])
                            t2 = rtmp.tile([128, 512], F32, tag="t2")
                            nc.vector.tensor_mul(t2[:], ps_[:], cos_sb[:, ts_])
                            nc.vector.tensor_add(dst[:, m, ts_], t1[:], t2[:])

                    # v token-major into v_aug
                    for tt in range(4):
                        tk_idx = 4 * s + tt
                        ps_ = psum.tile([128, 512], F32, tag="qkv")
                        for kk in range(KT):
                            nc.tensor.matmul(
                                ps_[:],
                                x_sb[:, kk, 128 * tt:128 * (tt + 1)],
                                w_sb["v"][:, kk, :],
                                start=(kk == 0),
                                stop=(kk == KT - 1),
                            )
                        nc.vector.tensor_copy(
                            v_aug[:, tk_idx, :, 0:64],
                            ps_.rearrange("p (h f) -> p h f", h=HPC),
                        )

                    if do_attn and s in (1, 2):
                        attn_chunk(nc, qT_sb, kT_sb, v_aug, tri_sb, y_dram,
                                   psum, probpool, ytmp, s)
                        do_ag(s)

            # ---- attention chunks 3 then 0 (rope-era pools closed above, so
            # the wo-phase pools below can coexist with these) ----
            if do_attn:
                attn_chunk(nc, qT_sb, kT_sb, v_aug, tri_sb, y_dram,
                           psum, probpool, ytmp, 3)
                do_ag(3)
                attn_chunk(nc, qT_sb, kT_sb, v_aug, tri_sb, y_dram,
                           psum, probpool, ytmp, 0)
                do_ag(0)

            # ---- token-parallel output projection ----
            if do_wo:
                with (
                    tc.tile_pool(name="wop", bufs=2) as wop,
                    tc.tile_pool(name="ypool", bufs=1) as ypool,
                    tc.tile_pool(name="otmp", bufs=2) as otmp,
                ):
                    oh_sb = ypool.tile([128, 4], F32)
                    nc.sync.dma_start(oh_sb[:], oh_dev[:])
                    yacc = ypool.tile([128, KT, 512], BF16)
                    for i, g in enumerate([1, 2, 3, 0]):  # chunk-arrival order
                        cand = ypool.tile([128, KT, 512], BF16, tag="cand",
                                          bufs=2, name=f"cand{g}")
                        nc.sync.dma_start(
                            cand[:], ag_dram[g].rearrange("(k p) t -> p k t", p=128)
                        )
                        # one-hot select, split across DVE / GpSimd halves
                        ks = slice(0, 10)
                        nc.vector.scalar_tensor_tensor(
                            yacc[:, ks, :], cand[:, ks, :], oh_sb[:, g:g + 1],
                            yacc[:, ks, :],
                            op0=mybir.AluOpType.mult,
                            op1=(mybir.AluOpType.bypass if i == 0
                                 else mybir.AluOpType.add),
                        )
                        ks = slice(10, 16)
                        gt = ypool.tile([128, 6, 512], BF16, tag="gsel", bufs=2)
                        nc.gpsimd.tensor_scalar_mul(
                            gt[:], cand[:, ks, :], oh_sb[:, g:g + 1]
                        )
                        if i == 0:
                            nc.gpsimd.tensor_copy(yacc[:, ks, :], gt[:])
                        else:
                            nc.gpsimd.tensor_add(yacc[:, ks, :], yacc[:, ks, :], gt[:])

                    for n in range(4):
                        wo_sb = wop.tile([128, KT, 512], BF16)
                        nc.sync.dma_start(
                            wo_sb[:], wo_dev[n].rearrange("p (k f) -> p k f", k=KT)
                        )
                        for m in range(4):
                            opsum = psum.tile([128, 512], F32, tag="qkv")
                            for kk in range(KT):
                                nc.tensor.matmul(
                                    opsum[:],
                                    yacc[:, kk, 128 * m:128 * (m + 1)],
                                    wo_sb[:, kk, :],
                                    start=(kk == 0),
                                    stop=(kk == KT - 1),
                                )
                            osb = otmp.tile([128, 512], F32)
                            nc.scalar.copy(osb[:], opsum[:])
                            nc.sync.dma_start(
                                out_dev[128 * m:128 * (m + 1),
                                        512 * n:512 * (n + 1)],
                                osb[:],
                            )

    nc.compile()
    return nc


def attn_chunk(nc, qT_sb, kT_sb, v_aug, tri_sb, y_dram, psum, probpool,
               ytmp, c):
    """Causal attention for query chunk c, all 8 heads, processed in pairs.

    The two heads of a pair sit in partition rows [0:64) and [64:128) of the
    same qT/kT feature tile, so their K=64 score matmuls land on different PE
    row-groups (tile_position auto-derived from base partitions) and execute
    concurrently. Each (128, 1024) scores psum holds one k-tile for BOTH
    heads; one exp covers both."""
    q0 = 512 * c
    cs = slice(q0, q0 + 512)
    ntk = 4 * (c + 1)
    for hp in range(4):
        hA, hB = 2 * hp, 2 * hp + 1
        ypsA = psum.tile([128, 512], F32, tag="y", name=f"ypsA_{c}_{hp}")
        ypsB = psum.tile([128, 512], F32, tag="y", name=f"ypsB_{c}_{hp}")
        for tk in range(ntk):
            ks = slice(128 * tk, 128 * (tk + 1))
            rel = max(128 * tk - q0, 0)  # first valid query column of this tile
            qs = slice(q0 + rel, q0 + 512)
            sAB = psum.tile([128, 1024], F32, tag="s2")
            nc.tensor.matmul(sAB[:, rel:512], kT_sb[0:64, hp, ks],
                             qT_sb[0:64, hp, qs], start=True, stop=True)
            nc.tensor.matmul(sAB[:, 512 + rel:1024], kT_sb[64:128, hp, ks],
                             qT_sb[64:128, hp, qs], start=True, stop=True)
            pr = probpool.tile([128, 1024], BF16, tag="pr")
            pr2 = pr.rearrange("p (h t) -> p h t", h=2)   # per-head halves
            s2v = sAB.rearrange("p (h t) -> p h t", h=2)
            if rel == 0 and 128 * tk - q0 < 0:
                nc.scalar.activation(
                    pr[:], sAB[:], mybir.ActivationFunctionType.Exp, scale=0.125,
                )
            else:
                # strided APs cover both heads' halves in one op each; columns
                # [0:rel) are never read by the PV matmuls below.
                nc.scalar.activation(
                    pr2[:, :, rel:512], s2v[:, :, rel:512],
                    mybir.ActivationFunctionType.Exp, scale=0.125,
                )
                nc.gpsimd.tensor_mul(
                    pr2[:, :, rel:rel + 128],
                    pr2[:, :, rel:rel + 128],
                    tri_sb.rearrange("p (h t) -> p h t", h=2),
                )
            first, last = tk == 0, tk == ntk - 1
            nc.tensor.matmul(ypsA[:, rel:512], v_aug[:, tk, hA, :],
                             pr[:, rel:512], start=first, stop=last)
            nc.tensor.matmul(ypsB[:, rel:512], v_aug[:, tk, hB, :],
                             pr[:, 512 + rel:1024], start=first, stop=last)

        for h, yps in ((hA, ypsA), (hB, ypsB)):
            recip = ytmp.tile([128, 512], F32, tag="recip")
            nc.vector.reciprocal(recip[64:128, :], yps[64:128, :])
            ynorm = ytmp.tile([64, 512], BF16, tag="ynorm")
            nc.vector.tensor_mul(ynorm[:], yps[0:64, :], recip[64:128, :])
            nc.sync.dma_start(y_dram[c][64 * h:64 * (h + 1), :], ynorm[:])


_NC_CACHE = {}


def _get_nc():
    if "nc" not in _NC_CACHE:
        _NC_CACHE["nc"] = build_nc()
    return _NC_CACHE["nc"]


def _pack_weightT(w):
    """(512, D) weight shard -> (128, KT*512) bf16 SBUF image of w.T k-tiles."""
    wT = np.ascontiguousarray(w.T)                       # (D, 512)
    r = wT.reshape(KT, 128, 512).transpose(1, 0, 2)      # (128, KT, 512)
    return np.ascontiguousarray(r.reshape(128, KT * 512).astype(bf16))


def make_in_maps(x, freqs_cos, freqs_sin, wq, wk, wv, wo):
    x = np.asarray(x, dtype=np.float32)
    freqs_cos = np.asarray(freqs_cos, dtype=np.float32)
    freqs_sin = np.asarray(freqs_sin, dtype=np.float32)
    wq, wk, wv, wo = (np.asarray(w, dtype=np.float32) for w in (wq, wk, wv, wo))

    # shared host-side packs
    p_idx = np.arange(128)
    j_idx = (p_idx % 64) // 2
    cosmap = freqs_cos.T[j_idx, :].astype(np.float32)               # (128, T)
    sinmap = (np.where(p_idx % 2 == 0, -1.0, 1.0)[:, None]
              * freqs_sin.T[j_idx, :]).astype(np.float32)
    tri1 = np.triu(np.ones((128, 128), np.float32))
    tri = np.concatenate([tri1, tri1], axis=1).astype(bf16)

    # x pack per batch: (NSLAB, 128, KT*512) where [s, p, kk*512 + t] = x[b, 512s+t, 128kk+p]
    x_pack = {}
    for b in range(B):
        xT = np.ascontiguousarray(x[b].T)                           # (D, T)
        r = xT.reshape(KT, 128, NSLAB, 512)                         # kk, p, s, t
        x_pack[b] = np.ascontiguousarray(
            r.transpose(2, 1, 0, 3).reshape(NSLAB, 128, KT * 512).astype(bf16)
        )

    # wo pack: (4, 128, KT*512) where [n, p, kk*512+f] = wo[512n+f, 128kk+p]
    woT = np.ascontiguousarray(wo.T)                                # (D, D) in-feat major
    r = woT.reshape(KT, 128, 4, 512)
    wo_pack = np.ascontiguousarray(
        r.transpose(2, 1, 0, 3).reshape(4, 128, KT * 512).astype(bf16)
    )

    in_maps = []
    for c in range(NCORES):
        b, g = divmod(c, 4)
        fs = slice(FPC * g, FPC * (g + 1))
        in_maps.append(
            {
                "x_dev": x_pack[b],
                "wq_dev": _pack_weightT(wq[fs]),
                "wk_dev": _pack_weightT(wk[fs]),
                "wv_dev": _pack_weightT(wv[fs]),
                "wo_dev": wo_pack,
                "cos_dev": cosmap,
                "sin_dev": sinmap,
                "tri_dev": tri,
                "oh_dev": np.tile(np.eye(4, dtype=np.float32)[g], (128, 1)),
            }
        )
    return in_maps


def kernel(x, freqs_cos, freqs_sin, wq, wk, wv, wo):
    nc = _get_nc()
    in_maps = make_in_maps(x, freqs_cos, freqs_sin, wq, wk, wv, wo)

    res = bass_utils.run_bass_kernel_spmd(
        nc, in_maps, core_ids=list(range(NCORES)), **_NC_CACHE.get("run_kwargs", {})
    )
    _NC_CACHE["last_result"] = res

    out = np.empty((B, T, D), dtype=np.float32)
    for c in range(NCORES):
        b, g = divmod(c, 4)
        out[b, 512 * g:512 * (g + 1), :] = res.results[c]["out_dev"]
    return out
